# revision 9
# baseline (speedup 1.0000x reference)
"""Multi-head attention (RoPE, causal) Trainium2 Bass kernel, 8-core SPMD.

Sharding: batch (2) x head-groups (4 heads/core). Per core:
  - QKV projections for its 4 heads (tensor-parallel over heads)
  - RoPE + causal attention for its 4 heads (2 head-pairs packed into
    128 partitions, exploiting causality at 128-key tile granularity)
  - AllGather of attention outputs within each 4-core batch group
  - Output projection sharded by output dim (per-core Wo slice)

All matmul layouts keep the contraction dim on partitions:
  Q^T/K^T: [dims, tok] straight out of the projection (x^T streamed as
  the moving operand), V: [tok, dims] (x^T as the stationary operand).
  Scores are computed transposed (S^T = K^T-stationary @ Q), softmax
  denominator via a ones-matmul on the PE, so no on-chip transposes of
  the attention matrix are ever needed.
"""

import functools
import os

import numpy as np

os.environ.setdefault("MYCRO_LOCAL_CACHE", "1")

D_MODEL = 1024
NUM_HEADS = 16
D_K = 64
THETA = 10000.0
B = 2
S = 2048
N_CORES = 8
GROUPS = [[0, 1, 2, 3], [4, 5, 6, 7]]
HPC = 4            # heads per core
DIMS = HPC * D_K   # 256 head-dims per core
OC = D_MODEL // 4  # 256 output dims per core (final projection)
NKT = S // 128     # 16 key tiles
NQC = S // 512     # 4 query chunks


def _build_nc(debug=False):
    from contextlib import ExitStack

    import concourse.tile as tile
    from concourse import bacc, mybir

    F32 = mybir.dt.float32
    EXP = mybir.ActivationFunctionType.Exp

    nc = bacc.Bacc(
        "TRN2",
        target_bir_lowering=False,
        debug=False,
        enable_asserts=False,
        num_devices=N_CORES,
    )

    xT_d = nc.dram_tensor("xT", [D_MODEL, S], F32, kind="ExternalInput")
    wqT_d = nc.dram_tensor("wqT", [D_MODEL, DIMS], F32, kind="ExternalInput")
    wkT_d = nc.dram_tensor("wkT", [D_MODEL, DIMS], F32, kind="ExternalInput")
    wvT_d = nc.dram_tensor("wvT", [D_MODEL, DIMS], F32, kind="ExternalInput")
    woT_d = nc.dram_tensor("woT", [D_MODEL, OC], F32, kind="ExternalInput")
    cos_d = nc.dram_tensor("cosT", [128, S], F32, kind="ExternalInput")
    sin_d = nc.dram_tensor("sinT", [128, S], F32, kind="ExternalInput")
    prot_d = nc.dram_tensor("prot", [128, 128], F32, kind="ExternalInput")
    tri_d = nc.dram_tensor("tri2", [128, 256], F32, kind="ExternalInput")
    out_d = nc.dram_tensor("out", [S, OC], F32, kind="ExternalOutput")

    bounce = nc.dram_tensor("bounce", [DIMS, S], F32)
    ag_out = nc.dram_tensor("ag_out", [D_MODEL, S], F32)
    if debug:
        qdump = nc.dram_tensor("qdump", [128, 2 * S], F32, kind="ExternalOutput")
        kdump = nc.dram_tensor("kdump", [128, 2 * S], F32, kind="ExternalOutput")
        vdump = nc.dram_tensor("vdump", [128, NKT, DIMS], F32, kind="ExternalOutput")
        aodump = nc.dram_tensor("aodump", [128, 2 * S], F32, kind="ExternalOutput")
        agdump = nc.dram_tensor("agdump", [D_MODEL, S], F32, kind="ExternalOutput")

    with tile.TileContext(nc) as tc, ExitStack() as ctx:
        const = ctx.enter_context(tc.tile_pool(name="const", bufs=1))

        wq_sb = const.tile([128, 8, DIMS], F32)
        wk_sb = const.tile([128, 8, DIMS], F32)
        wv_sb = const.tile([128, 8, DIMS], F32)
        cos_sb = const.tile([128, S], F32)
        sin_sb = const.tile([128, S], F32)
        prot_sb = const.tile([128, 128], F32)
        tri_sb = const.tile([128, 256], F32)
        ones_sb = const.tile([128, 64], F32)
        qp_sb = const.tile([128, 2 * S], F32)   # [dim-in-pair, pair*S + tok]
        kp_sb = const.tile([128, 2 * S], F32)
        v_sb = const.tile([128, NKT, DIMS], F32)  # [tok-in-tile, key-tile, dim]
        ao_sb = const.tile([128, 2 * S], F32)   # attention out, like qp_sb

        for d, t in ((wqT_d, wq_sb), (wkT_d, wk_sb), (wvT_d, wv_sb)):
            nc.sync.dma_start(t[:], d.ap().rearrange("(k p) m -> p k m", p=128))
        nc.sync.dma_start(cos_sb[:], cos_d[:])
        nc.sync.dma_start(sin_sb[:], sin_d[:])
        nc.sync.dma_start(prot_sb[:], prot_d[:])
        nc.sync.dma_start(tri_sb[:], tri_d[:])
        nc.gpsimd.memset(ones_sb[:], 1.0)

        # ---------------- Phase 1: QKV projections + RoPE ----------------
        with (
            tc.tile_pool(name="xin", bufs=2) as xpool,
            tc.tile_pool(name="p1sb", bufs=3) as p1sb,
            tc.tile_pool(name="p1qk", bufs=4, space="PSUM") as qkps,
            tc.tile_pool(name="p1rot", bufs=2, space="PSUM") as rotps,
            tc.tile_pool(name="p1v", bufs=2, space="PSUM") as vps,
        ):
            xT_r = xT_d.ap().rearrange("(k p) (c w) -> p k c w", p=128, w=512)
            for t in range(4):  # 512-token chunks
                x_sb = xpool.tile([128, 8, 512], F32, tag="x")
                nc.sync.dma_start(x_sb[:], xT_r[:, :, t, :])

                # Q/K projections ([dims, tok] layout) + RoPE
                for w_sb, dst in ((wq_sb, qp_sb), (wk_sb, kp_sb)):
                    for m in range(2):  # head-pair = partition tile of dims
                        ps = qkps.tile([128, 512], F32, tag="qk")
                        for k in range(8):
                            nc.tensor.matmul(
                                ps[:],
                                w_sb[:, k, 128 * m : 128 * m + 128],
                                x_sb[:, k, :],
                                start=(k == 0),
                                stop=(k == 7),
                            )
                        q_sb = p1sb.tile([128, 512], F32, tag="qsb")
                        nc.scalar.copy(q_sb[:], ps[:])
                        rps = rotps.tile([128, 512], F32, tag="rot")
                        nc.tensor.matmul(rps[:], prot_sb[:], q_sb[:], start=True, stop=True)
                        s1 = p1sb.tile([128, 512], F32, tag="s1")
                        nc.vector.tensor_mul(s1[:], rps[:], sin_sb[:, 512 * t : 512 * t + 512])
                        q2 = p1sb.tile([128, 512], F32, tag="q2")
                        nc.vector.tensor_mul(q2[:], q_sb[:], cos_sb[:, 512 * t : 512 * t + 512])
                        nc.gpsimd.tensor_add(
                            dst[:, S * m + 512 * t : S * m + 512 * t + 512], q2[:], s1[:]
                        )

                # V projection ([tok, dims] layout)
                for mt in range(4):
                    vp = vps.tile([128, DIMS], F32, tag="v")
                    for k in range(8):
                        nc.tensor.matmul(
                            vp[:],
                            x_sb[:, k, 128 * mt : 128 * mt + 128],
                            wv_sb[:, k, :],
                            start=(k == 0),
                            stop=(k == 7),
                        )
                    nc.scalar.copy(v_sb[:, 4 * t + mt, :], vp[:])

        # ---------------- Phase 2: causal attention ----------------
        with (
            tc.tile_pool(name="usb", bufs=4) as upool,
            tc.tile_pool(name="fin", bufs=2) as fpool,
            tc.tile_pool(name="sps", bufs=2, space="PSUM") as spool,
            tc.tile_pool(name="ops", bufs=2, space="PSUM") as opool,
            tc.tile_pool(name="dps", bufs=2, space="PSUM") as dpool,
        ):
            tri_v = tri_sb[:].rearrange("q (b w) -> q b w", b=2)
            for p in range(2):  # head pairs
                for c in range(NQC):  # 512-wide query chunks
                    outT = opool.tile([128, 512], F32, tag="o")
                    dacc = dpool.tile([128, 512], F32, tag="d")
                    nc.vector.memset(outT[:], 0.0)
                    nc.vector.memset(dacc[:], 0.0)
                    nk = 4 * c + 4
                    for k in range(nk):
                        jd = k - 4 * c  # >= 0 on the diagonal band
                        lo = 128 * jd if jd >= 0 else 0
                        w = 512 - lo
                        sp = spool.tile([128, 1024], F32, tag="s")
                        for h in (0, 1):
                            nc.tensor.matmul(
                                sp[:, 512 * h + lo : 512 * h + 512],
                                kp_sb[64 * h : 64 * h + 64, S * p + 128 * k : S * p + 128 * k + 128],
                                qp_sb[64 * h : 64 * h + 64, S * p + 512 * c + lo : S * p + 512 * c + 512],
                                start=True,
                                stop=True,
                            )
                        u = upool.tile([128, 1024], F32, tag="u")
                        u_v = u[:].rearrange("q (b w) -> q b w", b=2)
                        sp_v = sp[:].rearrange("q (b w) -> q b w", b=2)
                        nc.scalar.activation(
                            u_v[:, :, lo:512], sp_v[:, :, lo:512], EXP, scale=0.125
                        )
                        if jd >= 0:
                            nc.vector.tensor_mul(
                                u_v[:, :, lo : lo + 128], u_v[:, :, lo : lo + 128], tri_v
                            )
                        first, last = (k == 0), (k == nk - 1)
                        for h in (0, 1):
                            nc.tensor.matmul(
                                outT[64 * h : 64 * h + 64, lo:512],
                                v_sb[:, k, 128 * p + 64 * h : 128 * p + 64 * h + 64],
                                u[:, 512 * h + lo : 512 * h + 512],
                                start=False,
                                stop=last,
                                tile_position=(0, 64 * h),
                            )
                        for h in (0, 1):
                            nc.tensor.matmul(
                                dacc[64 * h : 64 * h + 64, lo:512],
                                ones_sb[:, 0:64],
                                u[:, 512 * h + lo : 512 * h + 512],
                                start=False,
                                stop=last,
                                tile_position=(0, 64 * h),
                            )
                    rcp = fpool.tile([128, 512], F32, tag="r")
                    nc.vector.reciprocal(rcp[:], dacc[:])
                    nc.vector.tensor_mul(
                        ao_sb[:, S * p + 512 * c : S * p + 512 * c + 512], outT[:], rcp[:]
                    )

        # ---------------- Phase 3: AllGather + output projection ----------------
        if debug:
            nc.sync.dma_start(qdump[:], qp_sb[:])
            nc.sync.dma_start(kdump[:], kp_sb[:])
            nc.sync.dma_start(vdump[:], v_sb[:])
            nc.sync.dma_start(aodump[:], ao_sb[:])
        nc.sync.dma_start(bounce[0:128, :], ao_sb[:, 0:S])
        nc.sync.dma_start(bounce[128:256, :], ao_sb[:, S : 2 * S])
        from concourse import mybir as _mb

        nc.gpsimd.collective_compute(
            "AllGather",
            _mb.AluOpType.bypass,
            ins=[bounce[:]],
            outs=[ag_out[:]],
            replica_groups=GROUPS,
        )

        with (
            tc.tile_pool(name="ph3", bufs=1) as wpool,
            tc.tile_pool(name="ph3o", bufs=2) as opool3,
            tc.tile_pool(name="ph3p", bufs=8, space="PSUM") as pps,
        ):
            wo_sb = wpool.tile([128, 8, OC], F32)
            nc.sync.dma_start(wo_sb[:], woT_d.ap().rearrange("(k p) m -> p k m", p=128))
            ag_sb = wpool.tile([128, 8, S], F32)
            ag_r = ag_out.ap().rearrange("(k p) t -> p k t", p=128)
            for k in range(8):
                nc.sync.dma_start(ag_sb[:, k, :], ag_r[:, k, :])
            if debug:
                nc.sync.dma_start(
                    agdump.ap().rearrange("(k p) t -> p k t", p=128), ag_sb[:]
                )
            out_sb = wpool.tile([128, 16, OC], F32)
            for g in range(2):  # groups of 8 token-tiles so psum fits
                pos = [
                    pps.tile([128, OC], F32, tag="po", name=f"po_{g}_{i}")
                    for i in range(8)
                ]
                for k in range(8):
                    for mt in range(8):
                        nc.tensor.matmul(
                            pos[mt][:],
                            ag_sb[:, k, 1024 * g + 128 * mt : 1024 * g + 128 * mt + 128],
                            wo_sb[:, k, :],
                            start=(k == 0),
                            stop=(k == 7),
                        )
                for mt in range(8):
                    nc.scalar.copy(out_sb[:, 8 * g + mt, :], pos[mt][:])
            nc.sync.dma_start(
                out_d.ap().rearrange("(mt pp) o -> pp mt o", pp=128), out_sb[:]
            )

    nc.compile()
    return nc


@functools.lru_cache(maxsize=1)
def _get_nc():
    return _build_nc()


def _host_inputs(x, Wq, Wk, Wv, Wo):
    """Host-side prep: per-core slices, transposes, RoPE tables."""
    f32 = np.float32
    inv_freq = (1.0 / (THETA ** (np.arange(0, D_K, 2, dtype=f32) / D_K))).astype(f32)
    t = np.arange(S, dtype=f32)
    freqs = t[:, None] * inv_freq[None, :]  # [S, 32]
    cos = np.cos(freqs).astype(f32)
    sin = np.sin(freqs).astype(f32)
    # [128, S] tables in [dim, tok] layout, periodic per 64 dims
    didx = (np.arange(128) % 64) // 2
    cosT = np.ascontiguousarray(cos[:, didx].T)
    sinT = np.ascontiguousarray(sin[:, didx].T)

    prot = np.zeros((128, 128), dtype=f32)
    g = np.arange(64)
    prot[2 * g + 1, 2 * g] = -1.0
    prot[2 * g, 2 * g + 1] = 1.0

    tri = (np.arange(128)[None, :] >= np.arange(128)[:, None]).astype(f32)
    tri2 = np.ascontiguousarray(np.tile(tri, (1, 2)))

    xT = [np.ascontiguousarray(x[b].T) for b in range(B)]
    woT = np.ascontiguousarray(Wo.T)

    in_maps = []
    for c in range(N_CORES):
        b, j = c // 4, c % 4
        in_maps.append(
            {
                "xT": xT[b],
                "wqT": np.ascontiguousarray(Wq[DIMS * j : DIMS * (j + 1), :].T),
                "wkT": np.ascontiguousarray(Wk[DIMS * j : DIMS * (j + 1), :].T),
                "wvT": np.ascontiguousarray(Wv[DIMS * j : DIMS * (j + 1), :].T),
                "woT": np.ascontiguousarray(woT[:, OC * j : OC * (j + 1)]),
                "cosT": cosT,
                "sinT": sinT,
                "prot": prot,
                "tri2": tri2,
            }
        )
    return in_maps


def _run(in_maps):
    from concourse.bass_utils import run_bass_kernel_spmd

    nc = _get_nc()
    return run_bass_kernel_spmd(nc, in_maps, core_ids=list(range(N_CORES))).results


def kernel(x, Wq, Wk, Wv, Wo):
    in_maps = _host_inputs(
        np.asarray(x), np.asarray(Wq), np.asarray(Wk), np.asarray(Wv), np.asarray(Wo)
    )
    results = _run(in_maps)
    out = np.empty((B, S, D_MODEL), dtype=np.float32)
    for c in range(N_CORES):
        b, j = c // 4, c % 4
        out[b, :, OC * j : OC * (j + 1)] = results[c]["out"]
    return out


# revision 17
# speedup vs baseline: 1409.8685x; 1409.8685x over previous
"""Multi-head attention (RoPE, causal) Trainium2 Bass kernel, 8-core SPMD.

Sharding: batch (2) x head-groups (4 heads/core). Per core:
  - QKV projections for its 4 heads (tensor-parallel over heads)
  - RoPE + causal attention for its 4 heads (2 head-pairs packed into
    128 partitions, causality exploited at 128-key tile granularity)
  - AllGather of attention outputs within each 4-core batch group
  - Output projection sharded by output dim (per-core Wo slice)

All matmuls run in float32r (full-rate fp32 PE streaming). Layouts keep
the contraction dim on partitions: Q^T/K^T come out of the projection as
[dims, tok] (x^T is the moving operand), V as [tok, dims] (x^T is the
stationary operand). Scores are computed transposed (S^T = K-stationary
@ Q) so softmax needs no on-chip transposes; the denominator rides the
PV matmul as a 65th ones-column of V, and the division is done via a
gpsimd partition-broadcast of the reciprocal row.
"""

import functools
import os

import numpy as np

os.environ.setdefault("MYCRO_LOCAL_CACHE", "1")

D_MODEL = 1024
NUM_HEADS = 16
D_K = 64
THETA = 10000.0
B = 2
S = 2048
N_CORES = 8
GROUPS = [[0, 1, 2, 3], [4, 5, 6, 7]]
HPC = 4            # heads per core
DIMS = HPC * D_K   # 256 head-dims per core
OC = D_MODEL // 4  # 256 output dims per core (final projection)
NKT = S // 128     # 16 key tiles
NQC = S // 512     # 4 query chunks


def _build_nc(debug=False, repeat=1, collective=True):
    from contextlib import ExitStack

    import concourse.tile as tile
    from concourse import bacc, mybir

    F32 = mybir.dt.float32
    F32R = mybir.dt.float32r
    EXP = mybir.ActivationFunctionType.Exp

    nc = bacc.Bacc(
        "TRN2",
        target_bir_lowering=False,
        debug=False,
        enable_asserts=False,
        num_devices=N_CORES,
    )

    xT_d = nc.dram_tensor("xT", [D_MODEL, S], F32R, kind="ExternalInput")
    wqT_d = nc.dram_tensor("wqT", [D_MODEL, DIMS], F32R, kind="ExternalInput")
    wkT_d = nc.dram_tensor("wkT", [D_MODEL, DIMS], F32R, kind="ExternalInput")
    wvT_d = nc.dram_tensor("wvT", [D_MODEL, DIMS], F32R, kind="ExternalInput")
    woT_d = nc.dram_tensor("woT", [D_MODEL, OC], F32R, kind="ExternalInput")
    cos_d = nc.dram_tensor("cosT", [128, S], F32, kind="ExternalInput")
    sin_d = nc.dram_tensor("sinT", [128, S], F32, kind="ExternalInput")
    prot_d = nc.dram_tensor("prot", [128, 128], F32R, kind="ExternalInput")
    tri_d = nc.dram_tensor("tri2", [128, 256], F32R, kind="ExternalInput")
    out_d = nc.dram_tensor("out", [S, OC], F32, kind="ExternalOutput")

    bounce_p = [
        nc.dram_tensor(f"bounce{p}", [128, S], F32R) for p in range(2)
    ]
    ag_out_p = [
        nc.dram_tensor(f"ag_out{p}", [512, S], F32R) for p in range(2)
    ]
    if debug:
        qdump = nc.dram_tensor("qdump", [128, 2 * S], F32, kind="ExternalOutput")
        kdump = nc.dram_tensor("kdump", [128, 2 * S], F32, kind="ExternalOutput")
        vdump = nc.dram_tensor("vdump", [128, NKT, 4, 65], F32, kind="ExternalOutput")
        aodump = nc.dram_tensor("aodump", [128, 2 * S], F32, kind="ExternalOutput")

    with tile.TileContext(nc) as tc, ExitStack() as ctx:
        # float32r is bit-identical fp32 data in the PE's full-rate
        # streaming mode; the low-precision guard misclassifies it.
        ctx.enter_context(nc.allow_low_precision(reason="float32r is fp32 data"))
        const = ctx.enter_context(tc.tile_pool(name="const", bufs=1))

        wq_sb = const.tile([128, 8, DIMS], F32R)
        wk_sb = const.tile([128, 8, DIMS], F32R)
        wv_sb = const.tile([128, 8, DIMS], F32R)
        cos_sb = const.tile([128, S], F32)
        sin_sb = const.tile([128, S], F32)
        prot_sb = const.tile([128, 128], F32R)
        tri_sb = const.tile([128, 256], F32R)
        qp_sb = const.tile([128, 2 * S], F32R)   # [dim-in-pair, pair*S + tok]
        kp_sb = const.tile([128, 2 * S], F32R)
        v_sb = const.tile([128, NKT, 4, 65], F32R)  # [tok, key-tile, head, dim|1]
        ao_sb = const.tile([128, 2 * S], F32R)   # attention out, like qp_sb

        for d, t in ((wqT_d, wq_sb), (wkT_d, wk_sb), (wvT_d, wv_sb)):
            nc.sync.dma_start(t[:], d.ap().rearrange("(k p) m -> p k m", p=128))
        nc.sync.dma_start(cos_sb[:], cos_d[:])
        nc.sync.dma_start(sin_sb[:], sin_d[:])
        nc.sync.dma_start(prot_sb[:], prot_d[:])
        nc.sync.dma_start(tri_sb[:], tri_d[:])
        wo_sb = const.tile([128, 8, OC], F32R)
        nc.sync.dma_start(wo_sb[:], woT_d.ap().rearrange("(k p) m -> p k m", p=128))
        ones64 = const.tile([128, 64], F32)
        nc.gpsimd.memset(ones64[:], 1.0)
        # denominator ones-column of V (f32r memset is illegal ISA; copy rounds)
        nc.scalar.copy(v_sb[:, :, :, 64], ones64[:])

        for rep in range(repeat):
            # ---------------- Phase 1: QKV projections + RoPE ----------------
            with (
                tc.tile_pool(name=f"xin{rep}", bufs=2) as xpool,
                tc.tile_pool(name=f"p1sb{rep}", bufs=3) as p1sb,
                tc.tile_pool(name=f"p1qk{rep}", bufs=4, space="PSUM") as qkps,
                tc.tile_pool(name=f"p1rot{rep}", bufs=2, space="PSUM") as rotps,
                tc.tile_pool(name=f"p1v{rep}", bufs=2, space="PSUM") as vps,
            ):
                xT_r = xT_d.ap().rearrange("(k p) (c w) -> p k c w", p=128, w=512)
                for t in range(4):  # 512-token chunks
                    x_sb = xpool.tile([128, 8, 512], F32R, tag="x")
                    for k in range(8):
                        nc.sync.dma_start(x_sb[:, k, :], xT_r[:, k, t, :])

                    # Q/K projections ([dims, tok] layout) + RoPE
                    for w_sb, dst in ((wq_sb, qp_sb), (wk_sb, kp_sb)):
                        for m in range(2):  # head-pair = partition tile of dims
                            ps = qkps.tile([128, 512], F32, tag="qk")
                            for k in range(8):
                                nc.tensor.matmul(
                                    ps[:],
                                    w_sb[:, k, 128 * m : 128 * m + 128],
                                    x_sb[:, k, :],
                                    start=(k == 0),
                                    stop=(k == 7),
                                )
                            q_sb = p1sb.tile([128, 512], F32R, tag="qsb")
                            nc.scalar.copy(q_sb[:], ps[:])
                            rps = rotps.tile([128, 512], F32, tag="rot")
                            nc.tensor.matmul(rps[:], prot_sb[:], q_sb[:], start=True, stop=True)
                            s1 = p1sb.tile([128, 512], F32, tag="s1")
                            nc.vector.tensor_mul(s1[:], rps[:], sin_sb[:, 512 * t : 512 * t + 512])
                            q2 = p1sb.tile([128, 512], F32, tag="q2")
                            nc.vector.tensor_mul(q2[:], q_sb[:], cos_sb[:, 512 * t : 512 * t + 512])
                            nc.gpsimd.tensor_add(
                                dst[:, S * m + 512 * t : S * m + 512 * t + 512], q2[:], s1[:]
                            )

                    # V projection ([tok, dims] layout)
                    for mt in range(4):
                        vp = vps.tile([128, DIMS], F32, tag="v")
                        for k in range(8):
                            nc.tensor.matmul(
                                vp[:],
                                x_sb[:, k, 128 * mt : 128 * mt + 128],
                                wv_sb[:, k, :],
                                start=(k == 0),
                                stop=(k == 7),
                            )
                        nc.vector.tensor_copy(
                            v_sb[:, 4 * t + mt, :, 0:64],
                            vp[:].rearrange("p (h d) -> p h d", d=64),
                        )

            # ---------------- Phase 2: causal attention ----------------
            with (
                tc.tile_pool(name=f"usb{rep}", bufs=4) as upool,
                tc.tile_pool(name=f"fin{rep}", bufs=3) as fpool,
                tc.tile_pool(name=f"sps{rep}", bufs=2, space="PSUM") as spool,
                tc.tile_pool(name=f"o0ps{rep}", bufs=2, space="PSUM") as o0pool,
                tc.tile_pool(name=f"o1ps{rep}", bufs=2, space="PSUM") as o1pool,
            ):
                tri_v = tri_sb[:].rearrange("q (b w) -> q b w", b=2)
                from concourse import mybir as _mb
                for p in range(2):  # head pairs
                    for c in range(NQC):  # 512-wide query chunks
                        o65 = [
                            o0pool.tile([65, 512], F32, tag="o0", name=f"o65a_{rep}_{p}_{c}"),
                            o1pool.tile([65, 512], F32, tag="o1", name=f"o65b_{rep}_{p}_{c}"),
                        ]
                        nk = 4 * c + 4
                        for k in range(nk):
                            jd = k - 4 * c  # >= 0 on the diagonal band
                            lo = 128 * jd if jd >= 0 else 0
                            sp = spool.tile([128, 1024], F32, tag="s")
                            for h in (0, 1):
                                nc.tensor.matmul(
                                    sp[:, 512 * h + lo : 512 * h + 512],
                                    kp_sb[64 * h : 64 * h + 64, S * p + 128 * k : S * p + 128 * k + 128],
                                    qp_sb[64 * h : 64 * h + 64, S * p + 512 * c + lo : S * p + 512 * c + 512],
                                    start=True,
                                    stop=True,
                                )
                            u = upool.tile([128, 1024], F32R, tag="u")
                            u_v = u[:].rearrange("q (b w) -> q b w", b=2)
                            sp_v = sp[:].rearrange("q (b w) -> q b w", b=2)
                            nc.scalar.activation(
                                u_v[:, :, lo:512], sp_v[:, :, lo:512], EXP, scale=0.125
                            )
                            if jd >= 0:
                                nc.vector.tensor_mul(
                                    u_v[:, :, lo : lo + 128], u_v[:, :, lo : lo + 128], tri_v
                                )
                            for h in (0, 1):
                                nc.tensor.matmul(
                                    o65[h][:, lo:512],
                                    v_sb[:, k, 2 * p + h, :],
                                    u[:, 512 * h + lo : 512 * h + 512],
                                    start=(k == 0),
                                    stop=(k == nk - 1),
                                )
                        for h in (0, 1):
                            rcp = fpool.tile([1, 512], F32R, tag="r", name=f"rcp_{rep}_{p}_{c}_{h}")
                            nc.vector.reciprocal(rcp[:], o65[h][64:65, :])
                            bcast = fpool.tile([64, 512], F32R, tag="b", name=f"bc_{rep}_{p}_{c}_{h}")
                            nc.gpsimd.partition_broadcast(bcast[:], rcp[:])
                            nc.vector.tensor_mul(
                                ao_sb[64 * h : 64 * h + 64, S * p + 512 * c : S * p + 512 * c + 512],
                                o65[h][0:64, :],
                                bcast[:],
                            )
                    # overlap this pair's AllGather with the next pair's attention
                    nc.sync.dma_start(bounce_p[p][:], ao_sb[:, S * p : S * p + S])
                    if collective:
                        nc.gpsimd.collective_compute(
                            "AllGather",
                            _mb.AluOpType.bypass,
                            ins=[bounce_p[p][:]],
                            outs=[ag_out_p[p][:]],
                            replica_groups=GROUPS,
                        )
                    else:
                        nc.sync.dma_start(ag_out_p[p][0:128, :], bounce_p[p][:])

            # ---------------- Phase 3: output projection ----------------
            if debug:
                nc.sync.dma_start(qdump[:], qp_sb[:].bitcast(F32))
                nc.sync.dma_start(kdump[:], kp_sb[:].bitcast(F32))
                nc.sync.dma_start(vdump[:], v_sb[:].bitcast(F32))
                nc.sync.dma_start(aodump[:], ao_sb[:].bitcast(F32))
            with (
                tc.tile_pool(name=f"ph3{rep}", bufs=1) as wpool,
                tc.tile_pool(name=f"ph3p{rep}", bufs=8, space="PSUM") as pps,
            ):
                ag_sb = wpool.tile([128, 8, S], F32R)
                ag_rs = [
                    ag_out_p[p].ap().rearrange("(k p2) t -> p2 k t", p2=128)
                    for p in range(2)
                ]
                for k in range(8):
                    nc.sync.dma_start(ag_sb[:, k, :], ag_rs[k // 4][:, k % 4, :])
                out_sb = wpool.tile([128, 16, OC], F32)
                for g in range(2):  # groups of 8 token-tiles so psum fits
                    pos = [
                        pps.tile([128, OC], F32, tag="po", name=f"po_{rep}_{g}_{i}")
                        for i in range(8)
                    ]
                    for k in range(8):
                        for mt in range(8):
                            nc.tensor.matmul(
                                pos[mt][:],
                                ag_sb[:, k, 1024 * g + 128 * mt : 1024 * g + 128 * mt + 128],
                                wo_sb[:, k, :],
                                start=(k == 0),
                                stop=(k == 7),
                            )
                    for mt in range(8):
                        nc.vector.tensor_copy(out_sb[:, 8 * g + mt, :], pos[mt][:])
                nc.sync.dma_start(
                    out_d.ap().rearrange("(mt pp) o -> pp mt o", pp=128), out_sb[:]
                )

    nc.compile()
    return nc


@functools.lru_cache(maxsize=4)
def _get_nc(repeat=1):
    return _build_nc(repeat=repeat)


def _host_inputs(x, Wq, Wk, Wv, Wo):
    """Host-side prep: per-core slices, transposes, RoPE tables."""
    f32 = np.float32
    inv_freq = (1.0 / (THETA ** (np.arange(0, D_K, 2, dtype=f32) / D_K))).astype(f32)
    t = np.arange(S, dtype=f32)
    freqs = t[:, None] * inv_freq[None, :]  # [S, 32]
    cos = np.cos(freqs).astype(f32)
    sin = np.sin(freqs).astype(f32)
    # [128, S] tables in [dim, tok] layout, periodic per 64 dims
    didx = (np.arange(128) % 64) // 2
    cosT = np.ascontiguousarray(cos[:, didx].T)
    sinT = np.ascontiguousarray(sin[:, didx].T)

    prot = np.zeros((128, 128), dtype=f32)
    g = np.arange(64)
    prot[2 * g + 1, 2 * g] = -1.0
    prot[2 * g, 2 * g + 1] = 1.0

    tri = (np.arange(128)[None, :] >= np.arange(128)[:, None]).astype(f32)
    tri2 = np.ascontiguousarray(np.tile(tri, (1, 2)))

    xT = [np.ascontiguousarray(x[b].T) for b in range(B)]
    # Wo.T rows permuted to match the two per-pair AllGather outputs:
    # AG-A rows = [core0 pair0 dims, core1 pair0, ...], then AG-B likewise.
    woT_n = Wo.T
    blocks = [woT_n[256 * i : 256 * i + 128, :] for i in range(4)] + [
        woT_n[256 * i + 128 : 256 * i + 256, :] for i in range(4)
    ]
    woT = np.ascontiguousarray(np.concatenate(blocks, axis=0))

    in_maps = []
    for c in range(N_CORES):
        b, j = c // 4, c % 4
        in_maps.append(
            {
                "xT": xT[b],
                "wqT": np.ascontiguousarray(Wq[DIMS * j : DIMS * (j + 1), :].T),
                "wkT": np.ascontiguousarray(Wk[DIMS * j : DIMS * (j + 1), :].T),
                "wvT": np.ascontiguousarray(Wv[DIMS * j : DIMS * (j + 1), :].T),
                "woT": np.ascontiguousarray(woT[:, OC * j : OC * (j + 1)]),
                "cosT": cosT,
                "sinT": sinT,
                "prot": prot,
                "tri2": tri2,
            }
        )
    return in_maps


@functools.lru_cache(maxsize=4)
def _get_exec(repeat=1):
    """Build the bass program once and return a persistent jitted callable.

    Mirrors concourse.bass2jax.run_bass_via_pjrt, but caches the jitted
    shard_map so repeated kernel() calls don't re-trace/re-compile.
    """
    import jax
    from jax.sharding import Mesh, PartitionSpec
    from jax.experimental.shard_map import shard_map

    from concourse import bass2jax, mybir

    nc = _get_nc(repeat)
    bass2jax.install_neuronx_cc_hook()

    partition_name = nc.partition_id_tensor.name if nc.partition_id_tensor else None
    in_names, out_names, out_avals = [], [], []
    for alloc in nc.m.functions[0].allocations:
        if not isinstance(alloc, mybir.MemoryLocationSet):
            continue
        name = alloc.memorylocations[0].name
        if alloc.kind == "ExternalInput":
            if name != partition_name:
                in_names.append(name)
        elif alloc.kind == "ExternalOutput":
            out_names.append(name)
            out_avals.append(
                jax.core.ShapedArray(
                    tuple(alloc.tensor_shape), mybir.dt.np(alloc.dtype)
                )
            )
    n_params = len(in_names)
    all_names = in_names + out_names
    if partition_name is not None:
        all_names = all_names + [partition_name]

    def _body(*args):
        operands = list(args)
        if partition_name is not None:
            operands.append(bass2jax.partition_id_tensor())
        return tuple(
            bass2jax._bass_exec_p.bind(
                *operands,
                out_avals=tuple(out_avals),
                in_names=tuple(all_names),
                out_names=tuple(out_names),
                lowering_input_output_aliases=(),
                sim_require_finite=True,
                sim_require_nnan=True,
                nc=nc,
            )
        )

    devices = jax.devices()[:N_CORES]
    mesh = Mesh(np.asarray(devices), ("core",))
    n_outs = len(out_names)
    donate = tuple(range(n_params, n_params + n_outs))
    sharded = jax.jit(
        shard_map(
            _body,
            mesh=mesh,
            in_specs=(PartitionSpec("core"),) * (n_params + n_outs),
            out_specs=(PartitionSpec("core"),) * n_outs,
            check_rep=False,
        ),
        donate_argnums=donate,
        keep_unused=True,
    )
    zero_protos = [
        (tuple((N_CORES * a.shape[0], *a.shape[1:])), a.dtype) for a in out_avals
    ]
    out_shapes = [tuple(a.shape) for a in out_avals]
    return sharded, in_names, out_names, n_params, zero_protos, out_shapes


def _concat_inputs(in_maps):
    _, in_names, _, _, _, _ = _get_exec()
    return [
        np.concatenate([np.asarray(in_maps[c][n]) for c in range(N_CORES)], axis=0)
        for n in in_names
    ]


def _exec(concat_in, as_numpy=True, repeat=1):
    sharded, _, out_names, _, zero_protos, out_shapes = _get_exec(repeat)
    zeros = [np.zeros(shape, dt) for shape, dt in zero_protos]
    out_arrs = sharded(*concat_in, *zeros)
    if not as_numpy:
        return out_arrs
    return [
        {
            n: np.asarray(out_arrs[i]).reshape(N_CORES, *out_shapes[i])[c]
            for i, n in enumerate(out_names)
        }
        for c in range(N_CORES)
    ]


def _run(in_maps):
    return _exec(_concat_inputs(in_maps))


def kernel(x, Wq, Wk, Wv, Wo):
    in_maps = _host_inputs(
        np.asarray(x), np.asarray(Wq), np.asarray(Wk), np.asarray(Wv), np.asarray(Wo)
    )
    results = _run(in_maps)
    out = np.empty((B, S, D_MODEL), dtype=np.float32)
    for c in range(N_CORES):
        b, j = c // 4, c % 4
        out[b, :, OC * j : OC * (j + 1)] = results[c]["out"]
    return out
